# Initial kernel scaffold
#
"""AttentionPoolingAggregator on 8 TRN2 NeuronCores (Bass/Tile).

Strategy (self-contained, shapes hardcoded):
  - Shard EDGES across the 8 cores by src-range: core i owns news rows
    [25000*i, 25000*(i+1)) and all edges whose src falls in that bank.
    Local news indices then fit int16 -> fast ucode dma_gather.
  - Replicate company_x / weights / v.  Each core computes
    company_proj = company_x @ W_company.T once (10016 rows).
  - Per 2048-edge batch: gather raw news rows G and company_proj rows B,
    transpose G on PE, a+b accumulated in PSUM (ACT copy b + 2 matmuls),
    tanh (ACT), score = sum(tanh * v) (DVE ttr), w = exp(score) (ACT),
    R = [w * G, w] (DVE), then dma_scatter_add R into a DRAM accumulator.
  - Scatter-add duplicates within one call race on HW, so the host
    precomputes conflict-free slots: idx = dst*3 + occ where occ is the
    occurrence rank of dst within the batch (occ<3), and overflow edges
    (occ>=3) get globally unique slots in a reserve region.  Calls are
    serialized by Tile, making cross-call read-modify-write safe.
  - Device then folds overflow slots back (gather + 3 scatter-add calls),
    folds the 3 occ slots per company, AllReduces the packed
    [10016, 257] partials across the 8 cores, and normalizes:
    out = num / max(den, 1e-9).
"""
import sys

sys.path.insert(0, "/opt/trn_rl_repo")

import numpy as np

N_NEWS = 200000
N_COMP = 10000
HID = 256
NCORES = 8
BANK = N_NEWS // NCORES  # 25000
SHARD = 81920
BATCH = 2048
NB = SHARD // BATCH  # 40
NGRP = BATCH // 128  # 16
K = 3
CPAD = 10016  # padded company count
NMAIN = K * CPAD  # 30048
OVF_BASE = NMAIN
NOVF = 1536
GARB = 31700  # garbage dump row
ACC_ROWS = 31744  # 248 * 128
ACC_W = 320
PACK_W = 257
NFOLD = 3  # overflow fold passes (handles up to 9 same-company overflows)

_compiled = None


def _build():
    import concourse.bacc as bacc
    import concourse.tile as tile
    import concourse.mybir as mybir
    from concourse.masks import make_identity

    f32 = mybir.dt.float32
    i16 = mybir.dt.int16
    AF = mybir.ActivationFunctionType
    ALU = mybir.AluOpType

    nc = bacc.Bacc("TRN2", target_bir_lowering=False, debug=False,
                   num_devices=NCORES)

    news_bank = nc.dram_tensor("news_bank", [BANK, HID], f32, kind="ExternalInput")
    cxT = nc.dram_tensor("cxT", [HID, CPAD], f32, kind="ExternalInput")
    WnT = nc.dram_tensor("WnT", [HID, HID], f32, kind="ExternalInput")
    WcT = nc.dram_tensor("WcT", [HID, HID], f32, kind="ExternalInput")
    v_rep = nc.dram_tensor("v_rep", [128, HID], f32, kind="ExternalInput")
    g_idx = nc.dram_tensor("g_idx", [128, SHARD // 16], i16, kind="ExternalInput")
    c_idx = nc.dram_tensor("c_idx", [128, SHARD // 16], i16, kind="ExternalInput")
    s_idx = nc.dram_tensor("s_idx", [128, SHARD // 16], i16, kind="ExternalInput")
    f_gidx = nc.dram_tensor("f_gidx", [128, NOVF // 16], i16, kind="ExternalInput")
    f_sidx = nc.dram_tensor("f_sidx", [128, NFOLD * NOVF // 16], i16,
                            kind="ExternalInput")
    out = nc.dram_tensor("out", [N_COMP, HID], f32, kind="ExternalOutput")

    CB = BATCH // 16  # idx columns per batch (128)

    with tile.TileContext(nc) as tc:
        with (
            tc.tile_pool(name="cst", bufs=1) as cst,
            tc.tile_pool(name="big", bufs=2) as big,
            tc.tile_pool(name="sm", bufs=3) as sm,
            tc.tile_pool(name="ps", bufs=2, space="PSUM") as ps,
            tc.tile_pool(name="dram", bufs=1, space="DRAM") as dp,
        ):
            cproj = dp.tile([CPAD, HID], f32)
            acc = dp.tile([ACC_ROWS, ACC_W], f32)
            packed = dp.tile([CPAD, PACK_W], f32)
            packed_sum = dp.tile([CPAD, PACK_W], f32, addr_space="Shared")

            # ---- constants / indices ----
            ident = cst.tile([128, 128], f32)
            make_identity(nc, ident[:])
            Wn0 = cst.tile([128, HID], f32)
            Wn1 = cst.tile([128, HID], f32)
            nc.sync.dma_start(Wn0[:], WnT[0:128, :])
            nc.sync.dma_start(Wn1[:], WnT[128:256, :])
            Wc0 = cst.tile([128, HID], f32)
            Wc1 = cst.tile([128, HID], f32)
            nc.sync.dma_start(Wc0[:], WcT[0:128, :])
            nc.sync.dma_start(Wc1[:], WcT[128:256, :])
            vb = cst.tile([128, HID], f32)
            nc.sync.dma_start(vb[:], v_rep[:])
            gi = cst.tile([128, SHARD // 16], i16)
            nc.sync.dma_start(gi[:], g_idx[:])
            ci = cst.tile([128, SHARD // 16], i16)
            nc.sync.dma_start(ci[:], c_idx[:])
            si = cst.tile([128, SHARD // 16], i16)
            nc.sync.dma_start(si[:], s_idx[:])
            fgi = cst.tile([128, NOVF // 16], i16)
            nc.sync.dma_start(fgi[:], f_gidx[:])
            fsi = cst.tile([128, NFOLD * NOVF // 16], i16)
            nc.sync.dma_start(fsi[:], f_sidx[:])

            # ---- zero the accumulator ----
            zt = cst.tile([128, 8 * ACC_W], f32)
            nc.vector.memset(zt[:], 0.0)
            acc_v = acc[:].rearrange("(a p) w -> p a w", p=128)  # [128, 248, 320]
            for z in range(31):
                nc.sync.dma_start(acc_v[:, 8 * z:8 * (z + 1), :],
                                  zt[:].rearrange("p (a w) -> p a w", w=ACC_W))

            # ---- company projection: cproj = company_x @ Wc.T ----
            for t in range(CPAD // 128):
                ct0 = sm.tile([128, 128], f32, tag="ct0")
                ct1 = sm.tile([128, 128], f32, tag="ct1")
                nc.sync.dma_start(ct0[:], cxT[0:128, 128 * t:128 * (t + 1)])
                nc.sync.dma_start(ct1[:], cxT[128:256, 128 * t:128 * (t + 1)])
                cp_ps = ps.tile([128, HID], f32, tag="cp")
                nc.tensor.matmul(cp_ps[:], lhsT=ct0[:], rhs=Wc0[:],
                                 start=True, stop=False)
                nc.tensor.matmul(cp_ps[:], lhsT=ct1[:], rhs=Wc1[:],
                                 start=False, stop=True)
                cp_sb = sm.tile([128, HID], f32, tag="cpsb")
                nc.scalar.copy(cp_sb[:], cp_ps[:])
                nc.sync.dma_start(cproj[128 * t:128 * (t + 1), :], cp_sb[:])

            # ---- edge batches ----
            for b in range(NB):
                gn = big.tile([128, NGRP, HID], f32, tag="gn")
                nc.gpsimd.dma_gather(
                    out_ap=gn[:], in_ap=news_bank[:],
                    idxs_ap=gi[:, CB * b:CB * (b + 1)],
                    num_idxs=BATCH, num_idxs_reg=BATCH, elem_size=HID)
                gc = big.tile([128, NGRP, HID], f32, tag="gc")
                nc.gpsimd.dma_gather(
                    out_ap=gc[:], in_ap=cproj[:],
                    idxs_ap=ci[:, CB * b:CB * (b + 1)],
                    num_idxs=BATCH, num_idxs_reg=BATCH, elem_size=HID)

                S = sm.tile([128, NGRP], f32, tag="S")
                for c in range(NGRP):
                    t01 = ps.tile([128, HID], f32, tag="t01")
                    nc.tensor.transpose(out=t01[:, 0:128], in_=gn[:, c, 0:128],
                                        identity=ident[:])
                    nc.tensor.transpose(out=t01[:, 128:256], in_=gn[:, c, 128:256],
                                        identity=ident[:])
                    gt = sm.tile([128, HID], f32, tag="gt")
                    nc.scalar.copy(gt[:], t01[:])
                    ab = ps.tile([128, HID], f32, tag="ab")
                    nc.scalar.copy(ab[:], gc[:, c, :])
                    nc.tensor.matmul(ab[:], lhsT=gt[:, 0:128], rhs=Wn0[:],
                                     start=False, stop=False)
                    nc.tensor.matmul(ab[:], lhsT=gt[:, 128:256], rhs=Wn1[:],
                                     start=False, stop=True)
                    Tt = sm.tile([128, HID], f32, tag="T")
                    nc.scalar.activation(Tt[:], ab[:], AF.Tanh)
                    scr = sm.tile([128, HID], f32, tag="scr")
                    nc.vector.tensor_tensor_reduce(
                        out=scr[:], in0=Tt[:], in1=vb[:], scale=1.0, scalar=0.0,
                        op0=ALU.mult, op1=ALU.add, accum_out=S[:, c:c + 1])

                WS = sm.tile([128, NGRP], f32, tag="WS")
                nc.scalar.activation(WS[:], S[:], AF.Exp)
                R = big.tile([128, NGRP, PACK_W], f32, tag="R")
                nc.vector.tensor_tensor(
                    out=R[:, :, 0:HID], in0=gn[:],
                    in1=WS[:].unsqueeze(2).to_broadcast([128, NGRP, HID]),
                    op=ALU.mult)
                nc.vector.tensor_copy(R[:, :, HID], WS[:])
                nc.gpsimd.dma_scatter_add(
                    out_ap=acc[:, 0:PACK_W], in_ap=R[:],
                    idxs_ap=si[:, CB * b:CB * (b + 1)],
                    num_idxs=BATCH, num_idxs_reg=BATCH,
                    elem_size=PACK_W, elem_step=ACC_W)

            # ---- fold overflow slots back into main K-slots ----
            govf = cst.tile([128, NOVF // 128, ACC_W], f32)
            nc.gpsimd.dma_gather(
                out_ap=govf[:], in_ap=acc[:], idxs_ap=fgi[:],
                num_idxs=NOVF, num_idxs_reg=NOVF, elem_size=ACC_W)
            for f in range(NFOLD):
                nc.gpsimd.dma_scatter_add(
                    out_ap=acc[:], in_ap=govf[:],
                    idxs_ap=fsi[:, (NOVF // 16) * f:(NOVF // 16) * (f + 1)],
                    num_idxs=NOVF, num_idxs_reg=NOVF,
                    elem_size=ACC_W, elem_step=ACC_W)

            # ---- fold the K occ-slots: packed[c] = sum_k acc[3c+k, :257] ----
            for t in range(CPAD // 128):
                nt = sm.tile([128, K, PACK_W], f32, tag="nt")
                nc.sync.dma_start(
                    nt[:],
                    acc[3 * 128 * t:3 * 128 * (t + 1), 0:PACK_W]
                    .rearrange("(c k) w -> c k w", k=K))
                na = sm.tile([128, PACK_W], f32, tag="na")
                nc.vector.tensor_add(na[:], nt[:, 0, :], nt[:, 1, :])
                nc.vector.tensor_add(na[:], na[:], nt[:, 2, :])
                nc.sync.dma_start(packed[128 * t:128 * (t + 1), :], na[:])

            # ---- all-reduce partials across the 8 cores ----
            nc.gpsimd.collective_compute(
                "AllReduce", mybir.AluOpType.add,
                replica_groups=[list(range(NCORES))],
                ins=[packed.opt()], outs=[packed_sum.opt()])

            # ---- normalize: out = num / max(den, 1e-9) ----
            for t in range(79):
                rows = min(128, N_COMP - 128 * t)
                pt = sm.tile([128, PACK_W], f32, tag="pt")
                nc.sync.dma_start(pt[:], packed_sum[128 * t:128 * (t + 1), :])
                dc = sm.tile([128, 1], f32, tag="dc")
                nc.vector.tensor_scalar_max(dc[:], pt[:, 256:257], 1e-9)
                rc = sm.tile([128, 1], f32, tag="rc")
                nc.vector.reciprocal(rc[:], dc[:])
                ot = sm.tile([128, HID], f32, tag="ot")
                nc.vector.tensor_scalar_mul(ot[:], pt[:, 0:HID], rc[:])
                nc.sync.dma_start(out[128 * t:128 * t + rows, :], ot[:rows, :])

    nc.compile()
    return nc


def _wrap16(idx):
    """idx [N] int -> [128, N//16] int16, j -> [j%16, j//16], replicated x8."""
    n = len(idx)
    a = np.ascontiguousarray(idx.reshape(n // 16, 16).T).astype(np.int16)
    return np.tile(a, (8, 1))


def _prep_core(src, dst, news_x, core):
    lo = BANK * core
    sel = (src >= lo) & (src < lo + BANK)
    s_loc = (src[sel] - lo).astype(np.int64)
    d = dst[sel].astype(np.int64)
    ne = len(d)
    assert ne <= SHARD, f"shard overflow: {ne}"
    s_pad = np.concatenate([s_loc, np.zeros(SHARD - ne, np.int64)])
    d_pad = np.concatenate([d, np.full(SHARD - ne, -1, np.int64)])

    scat = np.empty(SHARD, np.int64)
    ovf_dst = []
    for b in range(NB):
        db = d_pad[b * BATCH:(b + 1) * BATCH]
        order = np.argsort(db, kind="stable")
        sd = db[order]
        newgrp = np.r_[True, sd[1:] != sd[:-1]]
        grp_start = np.maximum.accumulate(np.where(newgrp, np.arange(BATCH), 0))
        rank_sorted = np.arange(BATCH) - grp_start
        occ = np.empty(BATCH, np.int64)
        occ[order] = rank_sorted
        sb = np.where(db < 0, GARB, db * K + np.minimum(occ, K - 1))
        ovf_mask = (occ >= K) & (db >= 0)
        for t in np.nonzero(ovf_mask)[0]:
            sb[t] = OVF_BASE + len(ovf_dst)
            ovf_dst.append(int(db[t]))
        scat[b * BATCH:(b + 1) * BATCH] = sb
    n_ovf = len(ovf_dst)
    assert n_ovf <= NOVF, f"overflow region too small: {n_ovf}"

    # fold indices: overflow slot k (company c_k) -> c_k*K + occp%K on pass occp//K
    fold = np.full((NFOLD, NOVF), GARB, np.int64)
    seen = {}
    for k, c in enumerate(ovf_dst):
        p = seen.get(c, 0)
        seen[c] = p + 1
        assert p < NFOLD * K, "too many same-company overflows"
        fold[p // K, k] = c * K + (p % K)

    return {
        "news_bank": np.ascontiguousarray(news_x[lo:lo + BANK]),
        "g_idx": _wrap16(s_pad),
        "c_idx": _wrap16(np.maximum(d_pad, 0)),
        "s_idx": _wrap16(scat),
        "f_sidx": np.concatenate([_wrap16(fold[f]) for f in range(NFOLD)], axis=1),
    }


def kernel(news_x, company_x, W_news, W_company, v, src, dst, num_companies):
    global _compiled
    from concourse import bass_utils

    news_x = np.asarray(news_x, dtype=np.float32)
    company_x = np.asarray(company_x, dtype=np.float32)
    W_news = np.asarray(W_news, dtype=np.float32)
    W_company = np.asarray(W_company, dtype=np.float32)
    v = np.asarray(v, dtype=np.float32).reshape(-1)
    src = np.asarray(src).astype(np.int64)
    dst = np.asarray(dst).astype(np.int64)

    assert news_x.shape == (N_NEWS, HID) and company_x.shape == (N_COMP, HID)
    assert int(num_companies) == N_COMP

    if _compiled is None:
        _compiled = _build()
    nc = _compiled

    cxT = np.zeros((HID, CPAD), np.float32)
    cxT[:, :N_COMP] = company_x.T
    shared = {
        "cxT": cxT,
        "WnT": np.ascontiguousarray(W_news.T),
        "WcT": np.ascontiguousarray(W_company.T),
        "v_rep": np.broadcast_to(v, (128, HID)).copy(),
        "f_gidx": _wrap16(np.arange(OVF_BASE, OVF_BASE + NOVF)),
    }
    in_maps = []
    for i in range(NCORES):
        m = dict(shared)
        m.update(_prep_core(src, dst, news_x, i))
        in_maps.append(m)

    res = bass_utils.run_bass_kernel_spmd(nc, in_maps, core_ids=list(range(NCORES)))
    return res.results[0]["out"]


# revision 15
# speedup vs baseline: 1.0122x; 1.0122x over previous
"""AttentionPoolingAggregator on 8 TRN2 NeuronCores (Bass/Tile).

Strategy (self-contained, shapes hardcoded):
  - Shard EDGES across the 8 cores by src-range: core i owns news rows
    [25000*i, 25000*(i+1)) and all edges whose src falls in that bank.
    Local news indices then fit int16 -> fast ucode dma_gather.
  - Replicate company_x / weights / v.  Each core computes
    company_proj = company_x @ W_company.T once (10016 rows).
  - Per 2048-edge batch: gather raw news rows G and company_proj rows B,
    transpose G on PE, a+b accumulated in PSUM (ACT copy b + 2 matmuls),
    tanh (ACT), score = sum(tanh * v) (DVE ttr), w = exp(score) (ACT),
    R = [w * G, w] (DVE), then dma_scatter_add R into a DRAM accumulator.
  - Scatter-add duplicates within one call race on HW, so the host
    precomputes conflict-free slots: idx = dst*3 + occ where occ is the
    occurrence rank of dst within the batch (occ<3), and overflow edges
    (occ>=3) get globally unique slots in a reserve region.  Calls are
    serialized by Tile, making cross-call read-modify-write safe.
  - Device then folds overflow slots back (gather + 3 scatter-add calls),
    folds the 3 occ slots per company, AllReduces the packed
    [10016, 257] partials across the 8 cores, and normalizes:
    out = num / max(den, 1e-9).
"""
import sys

sys.path.insert(0, "/opt/trn_rl_repo")

import numpy as np

N_NEWS = 200000
N_COMP = 10000
HID = 256
NCORES = 8
BANK = N_NEWS // NCORES  # 25000
SHARD = 81920
BATCH = 2048
NB = SHARD // BATCH  # 40
NGRP = BATCH // 128  # 16
K = 3
CPAD = 10112  # padded company count (multiple of 128)
NMAIN = K * CPAD  # 30336
OVF_BASE = NMAIN
NOVF = 1536  # overflow region 30336..31871
GARB = 31900  # garbage dump row
ACC_ROWS = 32000  # 250 * 128
ACC_W = 320
PACK_W = 257
NFOLD = 3  # overflow fold passes (handles up to 9 same-company overflows)

_compiled = None


def _build(n_batches=NB, with_fold=True, with_ar=True, dbg=False):
    import concourse.bacc as bacc
    import concourse.tile as tile
    import concourse.mybir as mybir
    from concourse.masks import make_identity

    f32 = mybir.dt.float32
    i16 = mybir.dt.int16
    AF = mybir.ActivationFunctionType
    ALU = mybir.AluOpType

    # NOTE: single_packet=False on >1024-idx gathers (HW packet ceiling is 64
    # descriptors); scatters are split into 1024-idx calls so their 2-desc/row
    # tx side fits the per-lane descriptor ring.
    # Two SWDGE queues: gathers on queue 1 (Q7 cores 2-3 + own descriptor
    # ring) overlap with scatters on queue 0 (cores 0-1) instead of
    # serializing through one ring.
    nc = bacc.Bacc("TRN2", target_bir_lowering=False, debug=False,
                   num_devices=NCORES, num_swdge_queues=2)

    news_bank = nc.dram_tensor("news_bank", [BANK, HID], f32, kind="ExternalInput")
    cxT = nc.dram_tensor("cxT", [HID, CPAD], f32, kind="ExternalInput")
    WnT = nc.dram_tensor("WnT", [HID, HID], f32, kind="ExternalInput")
    WcT = nc.dram_tensor("WcT", [HID, HID], f32, kind="ExternalInput")
    v_rep = nc.dram_tensor("v_rep", [128, HID], f32, kind="ExternalInput")
    g_idx = nc.dram_tensor("g_idx", [128, SHARD // 16], i16, kind="ExternalInput")
    c_idx = nc.dram_tensor("c_idx", [128, SHARD // 16], i16, kind="ExternalInput")
    s_idx = nc.dram_tensor("s_idx", [128, SHARD // 16], i16, kind="ExternalInput")
    f_gidx = nc.dram_tensor("f_gidx", [128, NOVF // 16], i16, kind="ExternalInput")
    f_sidx = nc.dram_tensor("f_sidx", [128, NFOLD * NOVF // 16], i16,
                            kind="ExternalInput")
    out = nc.dram_tensor("out", [N_COMP, HID], f32, kind="ExternalOutput")
    if dbg:
        dbg_acc = nc.dram_tensor("dbg_acc", [128, 4 * ACC_W], f32,
                                 kind="ExternalOutput")
        dbg_gn = nc.dram_tensor("dbg_gn", [128, NGRP * HID], f32,
                                kind="ExternalOutput")
        dbg_sw = nc.dram_tensor("dbg_sw", [128, 2 * NGRP], f32,
                                kind="ExternalOutput")
        dbg_r = nc.dram_tensor("dbg_r", [128, NGRP * PACK_W], f32,
                               kind="ExternalOutput")

    CB = BATCH // 16  # idx columns per batch (128)

    with tile.TileContext(nc) as tc:
        with (
            tc.tile_pool(name="cst", bufs=1) as cst,
            tc.tile_pool(name="big", bufs=2) as big,
            tc.tile_pool(name="sm", bufs=3) as sm,
            tc.tile_pool(name="ps", bufs=2, space="PSUM") as ps,
            tc.tile_pool(name="dram", bufs=1, space="DRAM") as dp,
        ):
            cproj = dp.tile([CPAD, HID], f32)
            acc = dp.tile([ACC_ROWS, ACC_W], f32)
            packed = dp.tile([CPAD, PACK_W], f32)
            packed_sum = dp.tile([CPAD, PACK_W], f32, addr_space="Shared")

            # ---- constants / indices ----
            ident = cst.tile([128, 128], f32)
            make_identity(nc, ident[:])
            Wn0 = cst.tile([128, HID], f32)
            Wn1 = cst.tile([128, HID], f32)
            nc.sync.dma_start(Wn0[:], WnT[0:128, :])
            nc.sync.dma_start(Wn1[:], WnT[128:256, :])
            Wc0 = cst.tile([128, HID], f32)
            Wc1 = cst.tile([128, HID], f32)
            nc.sync.dma_start(Wc0[:], WcT[0:128, :])
            nc.sync.dma_start(Wc1[:], WcT[128:256, :])
            vb = cst.tile([128, HID], f32)
            nc.sync.dma_start(vb[:], v_rep[:])
            gi = cst.tile([128, SHARD // 16], i16)
            nc.sync.dma_start(gi[:], g_idx[:])
            ci = cst.tile([128, SHARD // 16], i16)
            nc.sync.dma_start(ci[:], c_idx[:])
            si = cst.tile([128, SHARD // 16], i16)
            nc.sync.dma_start(si[:], s_idx[:])
            fgi = cst.tile([128, NOVF // 16], i16)
            nc.sync.dma_start(fgi[:], f_gidx[:])
            fsi = cst.tile([128, NFOLD * NOVF // 16], i16)
            nc.sync.dma_start(fsi[:], f_sidx[:])

            # ---- zero the accumulator ----
            zt = cst.tile([128, 10 * ACC_W], f32)
            nc.vector.memset(zt[:], 0.0)
            acc_v = acc[:].rearrange("(a p) w -> p a w", p=128)  # [128, 250, 320]
            for z in range(25):
                nc.sync.dma_start(acc_v[:, 10 * z:10 * (z + 1), :],
                                  zt[:, 0:10 * ACC_W].rearrange(
                                      "p (a w) -> p a w", w=ACC_W))

            # ---- company projection: cproj = company_x @ Wc.T ----
            for t in range(CPAD // 128):
                ct0 = sm.tile([128, 128], f32, tag="ct0")
                ct1 = sm.tile([128, 128], f32, tag="ct1")
                nc.sync.dma_start(ct0[:], cxT[0:128, 128 * t:128 * (t + 1)])
                nc.sync.dma_start(ct1[:], cxT[128:256, 128 * t:128 * (t + 1)])
                cp_ps = ps.tile([128, HID], f32, tag="cp")
                nc.tensor.matmul(cp_ps[:], lhsT=ct0[:], rhs=Wc0[:],
                                 start=True, stop=False)
                nc.tensor.matmul(cp_ps[:], lhsT=ct1[:], rhs=Wc1[:],
                                 start=False, stop=True)
                cp_sb = sm.tile([128, HID], f32, tag="cpsb")
                nc.scalar.copy(cp_sb[:], cp_ps[:])
                nc.sync.dma_start(cproj[128 * t:128 * (t + 1), :], cp_sb[:])

            # ---- edge batches ----
            for b in range(n_batches):
                gn = big.tile([128, NGRP, HID], f32, tag="gn")
                nc.gpsimd.dma_gather(
                    out_ap=gn[:], in_ap=news_bank[:],
                    idxs_ap=gi[:, CB * b:CB * (b + 1)],
                    num_idxs=BATCH, num_idxs_reg=BATCH, elem_size=HID,
                    single_packet=False, queue_num=1)
                gc = big.tile([128, NGRP, HID], f32, tag="gc")
                nc.gpsimd.dma_gather(
                    out_ap=gc[:], in_ap=cproj[:],
                    idxs_ap=ci[:, CB * b:CB * (b + 1)],
                    num_idxs=BATCH, num_idxs_reg=BATCH, elem_size=HID,
                    single_packet=False, queue_num=1)

                S = sm.tile([128, NGRP], f32, tag="S")
                for c in range(NGRP):
                    t01 = ps.tile([128, HID], f32, tag="t01")
                    nc.tensor.transpose(out=t01[:, 0:128], in_=gn[:, c, 0:128],
                                        identity=ident[:])
                    nc.tensor.transpose(out=t01[:, 128:256], in_=gn[:, c, 128:256],
                                        identity=ident[:])
                    gt = sm.tile([128, HID], f32, tag="gt")
                    nc.scalar.copy(gt[:], t01[:])
                    ab = ps.tile([128, HID], f32, tag="ab")
                    # PSUM groups must start with a matmul: copy b via identity
                    nc.tensor.matmul(ab[:], lhsT=ident[:], rhs=gc[:, c, :],
                                     start=True, stop=False)
                    nc.tensor.matmul(ab[:], lhsT=gt[:, 0:128], rhs=Wn0[:],
                                     start=False, stop=False)
                    nc.tensor.matmul(ab[:], lhsT=gt[:, 128:256], rhs=Wn1[:],
                                     start=False, stop=True)
                    Tt = sm.tile([128, HID], f32, tag="T")
                    nc.scalar.activation(Tt[:], ab[:], AF.Tanh)
                    scr = sm.tile([128, HID], f32, tag="scr")
                    nc.vector.tensor_tensor(out=scr[:], in0=Tt[:], in1=vb[:],
                                            op=ALU.mult)
                    nc.vector.tensor_reduce(S[:, c:c + 1], scr[:],
                                            axis=mybir.AxisListType.X, op=ALU.add)
                WS = sm.tile([128, NGRP], f32, tag="WS")
                nc.scalar.activation(WS[:], S[:], AF.Exp)
                R = big.tile([128, NGRP, PACK_W], f32, tag="R")
                nc.vector.tensor_tensor(
                    out=R[:, :, 0:HID], in0=gn[:],
                    in1=WS[:].unsqueeze(2).to_broadcast([128, NGRP, HID]),
                    op=ALU.mult)
                nc.vector.tensor_copy(R[:, :, HID], WS[:])
                if dbg and b == 0:
                    nc.sync.dma_start(dbg_gn[:], gn[:].rearrange("p a b -> p (a b)"))
                    nc.sync.dma_start(dbg_sw[:, 0:NGRP], S[:])
                    nc.sync.dma_start(dbg_sw[:, NGRP:2 * NGRP], WS[:])
                    nc.sync.dma_start(dbg_r[:], R[:].rearrange("p a b -> p (a b)"))
                half = NGRP // 2
                for h in range(2):
                    nc.gpsimd.dma_scatter_add(
                        out_ap=acc[:, 0:PACK_W],
                        in_ap=R[:, h * half:(h + 1) * half, :],
                        idxs_ap=si[:, CB * b + 64 * h:CB * b + 64 * (h + 1)],
                        num_idxs=BATCH // 2, num_idxs_reg=BATCH // 2,
                        elem_size=PACK_W, elem_step=ACC_W)

            if dbg:
                dtile = cst.tile([128, 4, ACC_W], f32)
                nc.sync.dma_start(
                    dtile[:],
                    acc[0:512, :].rearrange("(a p) w -> p a w", p=128))
                nc.sync.dma_start(dbg_acc[:],
                                  dtile[:].rearrange("p a w -> p (a w)"))

            # ---- fold overflow slots back into main K-slots ----
            govf = cst.tile([128, NOVF // 128, ACC_W], f32)
            if with_fold:
                nc.gpsimd.dma_gather(
                    out_ap=govf[:], in_ap=acc[:], idxs_ap=fgi[:],
                    num_idxs=NOVF, num_idxs_reg=NOVF, elem_size=ACC_W,
                    single_packet=False, queue_num=1)
            for f in range(NFOLD if with_fold else 0):
                for h in range(2):
                    nc.gpsimd.dma_scatter_add(
                        out_ap=acc[:],
                        in_ap=govf[:, (NOVF // 256) * h:(NOVF // 256) * (h + 1), :],
                        idxs_ap=fsi[:, (NOVF // 16) * f + (NOVF // 32) * h:
                                    (NOVF // 16) * f + (NOVF // 32) * (h + 1)],
                        num_idxs=NOVF // 2, num_idxs_reg=NOVF // 2,
                        elem_size=ACC_W, elem_step=ACC_W)

            # ---- fold the K occ-slots: packed[c] = sum_k acc[3c+k, :257] ----
            for t in range(CPAD // 128):
                nt = sm.tile([128, K, PACK_W], f32, tag="nt")
                nc.sync.dma_start(
                    nt[:],
                    acc[3 * 128 * t:3 * 128 * (t + 1), 0:PACK_W]
                    .rearrange("(c k) w -> c k w", k=K))
                na = sm.tile([128, PACK_W], f32, tag="na")
                nc.vector.tensor_add(na[:], nt[:, 0, :], nt[:, 1, :])
                nc.vector.tensor_add(na[:], na[:], nt[:, 2, :])
                nc.sync.dma_start(packed[128 * t:128 * (t + 1), :], na[:])

            # ---- all-reduce partials across the 8 cores ----
            if with_ar:
                nc.gpsimd.collective_compute(
                    "AllReduce", mybir.AluOpType.add,
                    replica_groups=[list(range(NCORES))],
                    ins=[packed.opt()], outs=[packed_sum.opt()])
            src_t = packed_sum if with_ar else packed

            # ---- normalize: out = num / max(den, 1e-9) ----
            for t in range(79):
                rows = min(128, N_COMP - 128 * t)
                prows = min(128, CPAD - 128 * t)
                pt = sm.tile([128, PACK_W], f32, tag="pt")
                nc.sync.dma_start(pt[:prows, :],
                                  src_t[128 * t:128 * t + prows, :])
                dc = sm.tile([128, 1], f32, tag="dc")
                nc.vector.tensor_scalar_max(dc[:prows], pt[:prows, 256:257], 1e-9)
                rc = sm.tile([128, 1], f32, tag="rc")
                nc.vector.reciprocal(rc[:prows], dc[:prows])
                ot = sm.tile([128, HID], f32, tag="ot")
                nc.vector.tensor_scalar_mul(ot[:prows], pt[:prows, 0:HID], rc[:prows])
                nc.sync.dma_start(out[128 * t:128 * t + rows, :], ot[:rows, :])

    nc.compile()
    return nc


def _wrap16(idx):
    """idx [N] int -> [128, N//16] int16, j -> [j%16, j//16], replicated x8."""
    n = len(idx)
    a = np.ascontiguousarray(idx.reshape(n // 16, 16).T).astype(np.int16)
    return np.tile(a, (8, 1))


def _prep_core(src, dst, news_x, core):
    lo = BANK * core
    sel = (src >= lo) & (src < lo + BANK)
    s_loc = (src[sel] - lo).astype(np.int64)
    d = dst[sel].astype(np.int64)
    ne = len(d)
    assert ne <= SHARD, f"shard overflow: {ne}"
    s_pad = np.concatenate([s_loc, np.zeros(SHARD - ne, np.int64)])
    d_pad = np.concatenate([d, np.full(SHARD - ne, -1, np.int64)])

    scat = np.empty(SHARD, np.int64)
    ovf_dst = []
    for b in range(NB):
        db = d_pad[b * BATCH:(b + 1) * BATCH]
        order = np.argsort(db, kind="stable")
        sd = db[order]
        newgrp = np.r_[True, sd[1:] != sd[:-1]]
        grp_start = np.maximum.accumulate(np.where(newgrp, np.arange(BATCH), 0))
        rank_sorted = np.arange(BATCH) - grp_start
        occ = np.empty(BATCH, np.int64)
        occ[order] = rank_sorted
        sb = np.where(db < 0, GARB, db * K + np.minimum(occ, K - 1))
        ovf_mask = (occ >= K) & (db >= 0)
        for t in np.nonzero(ovf_mask)[0]:
            sb[t] = OVF_BASE + len(ovf_dst)
            ovf_dst.append(int(db[t]))
        scat[b * BATCH:(b + 1) * BATCH] = sb
    n_ovf = len(ovf_dst)
    assert n_ovf <= NOVF, f"overflow region too small: {n_ovf}"

    # fold indices: overflow slot k (company c_k) -> c_k*K + occp%K on pass occp//K
    fold = np.full((NFOLD, NOVF), GARB, np.int64)
    seen = {}
    for k, c in enumerate(ovf_dst):
        p = seen.get(c, 0)
        seen[c] = p + 1
        assert p < NFOLD * K, "too many same-company overflows"
        fold[p // K, k] = c * K + (p % K)

    return {
        "news_bank": np.ascontiguousarray(news_x[lo:lo + BANK]),
        "g_idx": _wrap16(s_pad),
        "c_idx": _wrap16(np.maximum(d_pad, 0)),
        "s_idx": _wrap16(scat),
        "f_sidx": np.concatenate([_wrap16(fold[f]) for f in range(NFOLD)], axis=1),
    }


def kernel(news_x, company_x, W_news, W_company, v, src, dst, num_companies):
    global _compiled
    from concourse import bass_utils

    news_x = np.asarray(news_x, dtype=np.float32)
    company_x = np.asarray(company_x, dtype=np.float32)
    W_news = np.asarray(W_news, dtype=np.float32)
    W_company = np.asarray(W_company, dtype=np.float32)
    v = np.asarray(v, dtype=np.float32).reshape(-1)
    src = np.asarray(src).astype(np.int64)
    dst = np.asarray(dst).astype(np.int64)

    assert news_x.shape == (N_NEWS, HID) and company_x.shape == (N_COMP, HID)
    assert int(num_companies) == N_COMP

    if _compiled is None:
        _compiled = _build()
    nc = _compiled

    cxT = np.zeros((HID, CPAD), np.float32)
    cxT[:, :N_COMP] = company_x.T
    shared = {
        "cxT": cxT,
        "WnT": np.ascontiguousarray(W_news.T),
        "WcT": np.ascontiguousarray(W_company.T),
        "v_rep": np.broadcast_to(v, (128, HID)).copy(),
        "f_gidx": _wrap16(np.arange(OVF_BASE, OVF_BASE + NOVF)),
    }
    in_maps = []
    for i in range(NCORES):
        m = dict(shared)
        m.update(_prep_core(src, dst, news_x, i))
        in_maps.append(m)

    global _last_in_maps
    _last_in_maps = in_maps
    res = bass_utils.run_bass_kernel_spmd(nc, in_maps, core_ids=list(range(NCORES)))
    return res.results[0]["out"]


_last_in_maps = None


# revision 19
# speedup vs baseline: 1.0488x; 1.0362x over previous
"""AttentionPoolingAggregator on 8 TRN2 NeuronCores (Bass/Tile).

Strategy (self-contained, shapes hardcoded):
  - Shard EDGES across the 8 cores by src-range: core i owns news rows
    [25000*i, 25000*(i+1)) and all edges whose src falls in that bank.
    Local news indices then fit int16 -> fast ucode dma_gather.
  - Replicate company_x / weights / v.  Each core computes
    company_proj = company_x @ W_company.T once (10016 rows).
  - Per 2048-edge batch: gather raw news rows G and company_proj rows B,
    transpose G on PE, a+b accumulated in PSUM (ACT copy b + 2 matmuls),
    tanh (ACT), score = sum(tanh * v) (DVE ttr), w = exp(score) (ACT),
    R = [w * G, w] (DVE), then dma_scatter_add R into a DRAM accumulator.
  - Scatter-add duplicates within one call race on HW, so the host
    precomputes conflict-free slots: idx = dst*3 + occ where occ is the
    occurrence rank of dst within the batch (occ<3), and overflow edges
    (occ>=3) get globally unique slots in a reserve region.  Calls are
    serialized by Tile, making cross-call read-modify-write safe.
  - Device then folds overflow slots back (gather + 3 scatter-add calls),
    folds the 3 occ slots per company, AllReduces the packed
    [10016, 257] partials across the 8 cores, and normalizes:
    out = num / max(den, 1e-9).
"""
import sys

sys.path.insert(0, "/opt/trn_rl_repo")

import numpy as np

N_NEWS = 200000
N_COMP = 10000
HID = 256
NCORES = 8
BANK = N_NEWS // NCORES  # 25000
SHARD = 81920
BATCH = 2048
NB = SHARD // BATCH  # 40
NGRP = BATCH // 128  # 16
K = 3
CPAD = 10112  # padded company count (multiple of 128)
NMAIN = K * CPAD  # 30336
OVF_BASE = NMAIN
NOVF = 1536  # overflow region 30336..31871
GARB = 31900  # garbage dump row
ACC_ROWS = 32000  # 250 * 128
ACC_W = 320
PACK_W = 257
NFOLD = 3  # overflow fold passes (handles up to 9 same-company overflows)

_compiled = None


def _build(n_batches=NB, with_fold=True, with_ar=True, dbg=False):
    import concourse.bacc as bacc
    import concourse.tile as tile
    import concourse.mybir as mybir
    from concourse.masks import make_identity

    f32 = mybir.dt.float32
    i16 = mybir.dt.int16
    AF = mybir.ActivationFunctionType
    ALU = mybir.AluOpType

    # NOTE: single_packet=False on >1024-idx gathers (HW packet ceiling is 64
    # descriptors); scatters are split into 1024-idx calls so their 2-desc/row
    # tx side fits the per-lane descriptor ring.
    # Two SWDGE queues: gathers on queue 1 (Q7 cores 2-3 + own descriptor
    # ring) overlap with scatters on queue 0 (cores 0-1) instead of
    # serializing through one ring.
    nc = bacc.Bacc("TRN2", target_bir_lowering=False, debug=False,
                   num_devices=NCORES, num_swdge_queues=2,
                   dynamic_dma_scratch_size=32768)

    news_bank = nc.dram_tensor("news_bank", [BANK, HID], f32, kind="ExternalInput")
    cxT = nc.dram_tensor("cxT", [HID, CPAD], f32, kind="ExternalInput")
    WnT = nc.dram_tensor("WnT", [HID, HID], f32, kind="ExternalInput")
    WcT = nc.dram_tensor("WcT", [HID, HID], f32, kind="ExternalInput")
    v_rep = nc.dram_tensor("v_rep", [128, HID], f32, kind="ExternalInput")
    g_idx = nc.dram_tensor("g_idx", [128, SHARD // 16], i16, kind="ExternalInput")
    c_idx = nc.dram_tensor("c_idx", [128, SHARD // 16], i16, kind="ExternalInput")
    s_idx = nc.dram_tensor("s_idx", [128, SHARD // 16], i16, kind="ExternalInput")
    f_gidx = nc.dram_tensor("f_gidx", [128, NOVF // 16], i16, kind="ExternalInput")
    f_sidx = nc.dram_tensor("f_sidx", [128, NFOLD * NOVF // 16], i16,
                            kind="ExternalInput")
    out = nc.dram_tensor("out", [N_COMP, HID], f32, kind="ExternalOutput")
    if dbg:
        dbg_acc = nc.dram_tensor("dbg_acc", [128, 4 * ACC_W], f32,
                                 kind="ExternalOutput")
        dbg_gn = nc.dram_tensor("dbg_gn", [128, NGRP * HID], f32,
                                kind="ExternalOutput")
        dbg_sw = nc.dram_tensor("dbg_sw", [128, 2 * NGRP], f32,
                                kind="ExternalOutput")
        dbg_r = nc.dram_tensor("dbg_r", [128, NGRP * PACK_W], f32,
                               kind="ExternalOutput")

    CB = BATCH // 16  # idx columns per batch (128)

    with tile.TileContext(nc) as tc:
        with (
            tc.tile_pool(name="cst", bufs=1) as cst,
            tc.tile_pool(name="big", bufs=2) as big,
            tc.tile_pool(name="sm", bufs=3) as sm,
            tc.tile_pool(name="ps", bufs=3, space="PSUM") as ps,
            tc.tile_pool(name="ps2", bufs=2, space="PSUM") as ps2,
            tc.tile_pool(name="dram", bufs=1, space="DRAM") as dp,
        ):
            cproj = dp.tile([CPAD, HID], f32)
            acc = dp.tile([ACC_ROWS, ACC_W], f32)
            packed = dp.tile([CPAD, PACK_W], f32)
            packed_sum = dp.tile([CPAD, PACK_W], f32, addr_space="Shared")

            # ---- constants / indices ----
            ident = cst.tile([128, 128], f32)
            make_identity(nc, ident[:])
            Wn0 = cst.tile([128, HID], f32)
            Wn1 = cst.tile([128, HID], f32)
            nc.sync.dma_start(Wn0[:], WnT[0:128, :])
            nc.sync.dma_start(Wn1[:], WnT[128:256, :])
            Wc0 = cst.tile([128, HID], f32)
            Wc1 = cst.tile([128, HID], f32)
            nc.sync.dma_start(Wc0[:], WcT[0:128, :])
            nc.sync.dma_start(Wc1[:], WcT[128:256, :])
            vb = cst.tile([128, HID], f32)
            nc.sync.dma_start(vb[:], v_rep[:])
            gi = cst.tile([128, SHARD // 16], i16)
            nc.sync.dma_start(gi[:], g_idx[:])
            ci = cst.tile([128, SHARD // 16], i16)
            nc.sync.dma_start(ci[:], c_idx[:])
            si = cst.tile([128, SHARD // 16], i16)
            nc.sync.dma_start(si[:], s_idx[:])
            fgi = cst.tile([128, NOVF // 16], i16)
            nc.sync.dma_start(fgi[:], f_gidx[:])
            fsi = cst.tile([128, NFOLD * NOVF // 16], i16)
            nc.sync.dma_start(fsi[:], f_sidx[:])

            # ---- zero the accumulator ----
            zt = cst.tile([128, 5 * ACC_W], f32)
            nc.vector.memset(zt[:], 0.0)
            acc_v = acc[:].rearrange("(a p) w -> p a w", p=128)  # [128, 250, 320]
            for z in range(50):
                nc.sync.dma_start(acc_v[:, 5 * z:5 * (z + 1), :],
                                  zt[:, 0:5 * ACC_W].rearrange(
                                      "p (a w) -> p a w", w=ACC_W))

            # ---- company projection: cproj = company_x @ Wc.T ----
            for t in range(CPAD // 128):
                ct0 = sm.tile([128, 128], f32, tag="ct0")
                ct1 = sm.tile([128, 128], f32, tag="ct1")
                nc.sync.dma_start(ct0[:], cxT[0:128, 128 * t:128 * (t + 1)])
                nc.sync.dma_start(ct1[:], cxT[128:256, 128 * t:128 * (t + 1)])
                cp_ps = ps2.tile([128, HID], f32, tag="cp")
                nc.tensor.matmul(cp_ps[:], lhsT=ct0[:], rhs=Wc0[:],
                                 start=True, stop=False)
                nc.tensor.matmul(cp_ps[:], lhsT=ct1[:], rhs=Wc1[:],
                                 start=False, stop=True)
                cp_sb = sm.tile([128, HID], f32, tag="cpsb")
                nc.scalar.copy(cp_sb[:], cp_ps[:])
                nc.sync.dma_start(cproj[128 * t:128 * (t + 1), :], cp_sb[:])

            # ---- edge batches ----
            for b in range(n_batches):
                gn = big.tile([128, NGRP, HID], f32, tag="gn")
                nc.gpsimd.dma_gather(
                    out_ap=gn[:], in_ap=news_bank[:],
                    idxs_ap=gi[:, CB * b:CB * (b + 1)],
                    num_idxs=BATCH, num_idxs_reg=BATCH, elem_size=HID,
                    single_packet=False, queue_num=1)
                gc = big.tile([128, NGRP, HID], f32, tag="gc")
                nc.gpsimd.dma_gather(
                    out_ap=gc[:], in_ap=cproj[:],
                    idxs_ap=ci[:, CB * b:CB * (b + 1)],
                    num_idxs=BATCH, num_idxs_reg=BATCH, elem_size=HID,
                    single_packet=False, queue_num=1)

                S = sm.tile([128, NGRP], f32, tag="S")
                for c in range(NGRP):
                    t01 = ps.tile([128, HID], f32, tag="t01")
                    nc.tensor.transpose(out=t01[:, 0:128], in_=gn[:, c, 0:128],
                                        identity=ident[:])
                    nc.tensor.transpose(out=t01[:, 128:256], in_=gn[:, c, 128:256],
                                        identity=ident[:])
                    gt = sm.tile([128, HID], f32, tag="gt")
                    nc.scalar.copy(gt[:], t01[:])
                    ab = ps.tile([128, HID], f32, tag="ab")
                    # PSUM groups must start with a matmul: copy b via identity
                    nc.tensor.matmul(ab[:], lhsT=ident[:], rhs=gc[:, c, :],
                                     start=True, stop=False)
                    nc.tensor.matmul(ab[:], lhsT=gt[:, 0:128], rhs=Wn0[:],
                                     start=False, stop=False)
                    nc.tensor.matmul(ab[:], lhsT=gt[:, 128:256], rhs=Wn1[:],
                                     start=False, stop=True)
                    Tt = sm.tile([128, HID], f32, tag="T")
                    nc.scalar.activation(Tt[:], ab[:], AF.Tanh)
                    scr = sm.tile([128, HID], f32, tag="scr")
                    nc.vector.tensor_tensor(out=scr[:], in0=Tt[:], in1=vb[:],
                                            op=ALU.mult)
                    nc.vector.tensor_reduce(S[:, c:c + 1], scr[:],
                                            axis=mybir.AxisListType.X, op=ALU.add)
                WS = sm.tile([128, NGRP], f32, tag="WS")
                nc.scalar.activation(WS[:], S[:], AF.Exp)
                R = big.tile([128, NGRP, PACK_W], f32, tag="R")
                nc.vector.tensor_tensor(
                    out=R[:, :, 0:HID], in0=gn[:],
                    in1=WS[:].unsqueeze(2).to_broadcast([128, NGRP, HID]),
                    op=ALU.mult)
                nc.vector.tensor_copy(R[:, :, HID], WS[:])
                if dbg and b == 0:
                    nc.sync.dma_start(dbg_gn[:], gn[:].rearrange("p a b -> p (a b)"))
                    nc.sync.dma_start(dbg_sw[:, 0:NGRP], S[:])
                    nc.sync.dma_start(dbg_sw[:, NGRP:2 * NGRP], WS[:])
                    nc.sync.dma_start(dbg_r[:], R[:].rearrange("p a b -> p (a b)"))
                half = NGRP // 2
                for h in range(2):
                    nc.gpsimd.dma_scatter_add(
                        out_ap=acc[:, 0:PACK_W],
                        in_ap=R[:, h * half:(h + 1) * half, :],
                        idxs_ap=si[:, CB * b + 64 * h:CB * b + 64 * (h + 1)],
                        num_idxs=BATCH // 2, num_idxs_reg=BATCH // 2,
                        elem_size=PACK_W, elem_step=ACC_W)

            if dbg:
                dtile = cst.tile([128, 4, ACC_W], f32)
                nc.sync.dma_start(
                    dtile[:],
                    acc[0:512, :].rearrange("(a p) w -> p a w", p=128))
                nc.sync.dma_start(dbg_acc[:],
                                  dtile[:].rearrange("p a w -> p (a w)"))

            # ---- fold overflow slots back into main K-slots ----
            govf = cst.tile([128, NOVF // 128, ACC_W], f32)
            if with_fold:
                nc.gpsimd.dma_gather(
                    out_ap=govf[:], in_ap=acc[:], idxs_ap=fgi[:],
                    num_idxs=NOVF, num_idxs_reg=NOVF, elem_size=ACC_W,
                    single_packet=False, queue_num=1)
            for f in range(NFOLD if with_fold else 0):
                for h in range(2):
                    nc.gpsimd.dma_scatter_add(
                        out_ap=acc[:],
                        in_ap=govf[:, (NOVF // 256) * h:(NOVF // 256) * (h + 1), :],
                        idxs_ap=fsi[:, (NOVF // 16) * f + (NOVF // 32) * h:
                                    (NOVF // 16) * f + (NOVF // 32) * (h + 1)],
                        num_idxs=NOVF // 2, num_idxs_reg=NOVF // 2,
                        elem_size=ACC_W, elem_step=ACC_W)

            # ---- fold the K occ-slots: packed[c] = sum_k acc[3c+k, :257] ----
            for t in range(CPAD // 128):
                nt = sm.tile([128, K, PACK_W], f32, tag="nt")
                nc.sync.dma_start(
                    nt[:],
                    acc[3 * 128 * t:3 * 128 * (t + 1), 0:PACK_W]
                    .rearrange("(c k) w -> c k w", k=K))
                na = sm.tile([128, PACK_W], f32, tag="na")
                nc.vector.tensor_add(na[:], nt[:, 0, :], nt[:, 1, :])
                nc.vector.tensor_add(na[:], na[:], nt[:, 2, :])
                nc.sync.dma_start(packed[128 * t:128 * (t + 1), :], na[:])

            # ---- all-reduce partials across the 8 cores ----
            if with_ar:
                nc.gpsimd.collective_compute(
                    "AllReduce", mybir.AluOpType.add,
                    replica_groups=[list(range(NCORES))],
                    ins=[packed.opt()], outs=[packed_sum.opt()])
            src_t = packed_sum if with_ar else packed

            # ---- normalize: out = num / max(den, 1e-9) ----
            for t in range(79):
                rows = min(128, N_COMP - 128 * t)
                prows = min(128, CPAD - 128 * t)
                pt = sm.tile([128, PACK_W], f32, tag="pt")
                nc.sync.dma_start(pt[:prows, :],
                                  src_t[128 * t:128 * t + prows, :])
                dc = sm.tile([128, 1], f32, tag="dc")
                nc.vector.tensor_scalar_max(dc[:prows], pt[:prows, 256:257], 1e-9)
                rc = sm.tile([128, 1], f32, tag="rc")
                nc.vector.reciprocal(rc[:prows], dc[:prows])
                ot = sm.tile([128, HID], f32, tag="ot")
                nc.vector.tensor_scalar_mul(ot[:prows], pt[:prows, 0:HID], rc[:prows])
                nc.sync.dma_start(out[128 * t:128 * t + rows, :], ot[:rows, :])

    nc.compile()
    return nc


def _wrap16(idx):
    """idx [N] int -> [128, N//16] int16, j -> [j%16, j//16], replicated x8."""
    n = len(idx)
    a = np.ascontiguousarray(idx.reshape(n // 16, 16).T).astype(np.int16)
    return np.tile(a, (8, 1))


def _prep_core(src, dst, news_x, core):
    lo = BANK * core
    sel = (src >= lo) & (src < lo + BANK)
    s_loc = (src[sel] - lo).astype(np.int64)
    d = dst[sel].astype(np.int64)
    ne = len(d)
    assert ne <= SHARD, f"shard overflow: {ne}"
    s_pad = np.concatenate([s_loc, np.zeros(SHARD - ne, np.int64)])
    d_pad = np.concatenate([d, np.full(SHARD - ne, -1, np.int64)])

    scat = np.empty(SHARD, np.int64)
    ovf_dst = []
    for b in range(NB):
        db = d_pad[b * BATCH:(b + 1) * BATCH]
        order = np.argsort(db, kind="stable")
        sd = db[order]
        newgrp = np.r_[True, sd[1:] != sd[:-1]]
        grp_start = np.maximum.accumulate(np.where(newgrp, np.arange(BATCH), 0))
        rank_sorted = np.arange(BATCH) - grp_start
        occ = np.empty(BATCH, np.int64)
        occ[order] = rank_sorted
        sb = np.where(db < 0, GARB, db * K + np.minimum(occ, K - 1))
        ovf_mask = (occ >= K) & (db >= 0)
        for t in np.nonzero(ovf_mask)[0]:
            sb[t] = OVF_BASE + len(ovf_dst)
            ovf_dst.append(int(db[t]))
        scat[b * BATCH:(b + 1) * BATCH] = sb
    n_ovf = len(ovf_dst)
    assert n_ovf <= NOVF, f"overflow region too small: {n_ovf}"

    # fold indices: overflow slot k (company c_k) -> c_k*K + occp%K on pass occp//K
    fold = np.full((NFOLD, NOVF), GARB, np.int64)
    seen = {}
    for k, c in enumerate(ovf_dst):
        p = seen.get(c, 0)
        seen[c] = p + 1
        assert p < NFOLD * K, "too many same-company overflows"
        fold[p // K, k] = c * K + (p % K)

    return {
        "news_bank": np.ascontiguousarray(news_x[lo:lo + BANK]),
        "g_idx": _wrap16(s_pad),
        "c_idx": _wrap16(np.maximum(d_pad, 0)),
        "s_idx": _wrap16(scat),
        "f_sidx": np.concatenate([_wrap16(fold[f]) for f in range(NFOLD)], axis=1),
    }


def kernel(news_x, company_x, W_news, W_company, v, src, dst, num_companies):
    global _compiled
    from concourse import bass_utils

    news_x = np.asarray(news_x, dtype=np.float32)
    company_x = np.asarray(company_x, dtype=np.float32)
    W_news = np.asarray(W_news, dtype=np.float32)
    W_company = np.asarray(W_company, dtype=np.float32)
    v = np.asarray(v, dtype=np.float32).reshape(-1)
    src = np.asarray(src).astype(np.int64)
    dst = np.asarray(dst).astype(np.int64)

    assert news_x.shape == (N_NEWS, HID) and company_x.shape == (N_COMP, HID)
    assert int(num_companies) == N_COMP

    if _compiled is None:
        _compiled = _build()
    nc = _compiled

    cxT = np.zeros((HID, CPAD), np.float32)
    cxT[:, :N_COMP] = company_x.T
    shared = {
        "cxT": cxT,
        "WnT": np.ascontiguousarray(W_news.T),
        "WcT": np.ascontiguousarray(W_company.T),
        "v_rep": np.broadcast_to(v, (128, HID)).copy(),
        "f_gidx": _wrap16(np.arange(OVF_BASE, OVF_BASE + NOVF)),
    }
    in_maps = []
    for i in range(NCORES):
        m = dict(shared)
        m.update(_prep_core(src, dst, news_x, i))
        in_maps.append(m)

    global _last_in_maps
    _last_in_maps = in_maps
    res = bass_utils.run_bass_kernel_spmd(nc, in_maps, core_ids=list(range(NCORES)))
    return res.results[0]["out"]


_last_in_maps = None
